# revision 14
# baseline (speedup 1.0000x reference)
"""DiffuseEnhancer (GNN mean-aggregation + gated MLP + LayerNorm) on 8 TRN2
NeuronCores via Bass/Tile.

Strategy (SPMD, one program for all 8 cores):
- Nodes sharded by destination: core c owns dst rows [c*12500, (c+1)*12500).
- Edges partitioned by destination core; per core, grouped by 128-dst
  segments. Edge-source features are DMA-gathered (dma_gather, int16
  indices) from a per-core compacted bf16 node table: the core's unique
  source nodes, split into two <=32768-row buckets so indices fit int16.
- Gathers run on 4 SWDGE queues (num_swdge_queues=4) as 1024-row
  single-packet sub-gathers striped round-robin over queue_num 0-3;
  this roughly halves the Q7 descriptor-emission wall (~7.6 -> ~4.2
  ns/row) that otherwise serializes the whole kernel.
- Mean aggregation per segment via TensorE: host-built one-hot S
  matrices ([slots, 128 dst] bf16, streamed from DRAM per group) times
  gathered features, accumulated in PSUM -> msg[128 dst x 128 feat].
  Streaming S replaces the on-device is_equal build, whose broadcast
  operands forced DVE 1x mode (~103us).
- Epilogue per segment fuses: mean-scale + subtract (scalar_tensor_tensor,
  reads PSUM), squared-norm (ACT Square + accum), tanh gate, bottleneck
  MLP (two matmuls), residual assembly, LayerNorm (bn_stats/bn_aggr).

The tile/bucket schedule is shared across cores (max over cores, padded
slots gather throwaway rows and carry all-zero S columns), so a single
NEFF serves all 8 cores; per-core data lives in the input tensors.
"""

import os
import sys

for _p in ("/opt/trn_rl_repo", "/root/.axon_site/_ro/trn_rl_repo"):
    if os.path.isdir(_p) and _p not in sys.path:
        sys.path.insert(0, _p)

import numpy as np
import ml_dtypes

# graceful degradation if the NTFF profile hook module is absent
try:
    import antenv.axon_hooks  # noqa: F401
except ImportError:
    import types

    _m = types.ModuleType("antenv.axon_hooks")
    _m._HOOK = None
    _m.set_axon_ntff_profile_hook = lambda h: setattr(_m, "_HOOK", h)
    _m.get_axon_ntff_profile_hook = lambda: _m._HOOK
    sys.modules["antenv.axon_hooks"] = _m

import concourse.bass as bass
import concourse.bacc as bacc
import concourse.tile as tile
from concourse import mybir
from concourse.bass_utils import run_bass_kernel_spmd
from concourse.vector_clock import ScopedClock

ALPHA = 0.2
LN_EPS = 1e-5

N, D, C = 100000, 128, 8
P = N // C            # 12500 nodes per core
SEG = 128
NSEG = (P + SEG - 1) // SEG       # 98
PPAD = NSEG * SEG                 # 12544
NB = 2                            # src buckets per core
BCUT = 32768                      # bucket A = first 32768 unique srcs
TABLE_ROWS = 2 * BCUT             # fixed per-core gather table height
GSEG = 7                          # segments per gather/epilogue group
NG = NSEG // GSEG                 # 14
GROWS = GSEG * SEG                # 1792
MM1_CHUNK = 512
SENTINEL = 255.0

BF16 = mybir.dt.bfloat16
F32 = mybir.dt.float32
I16 = mybir.dt.int16


def _install_drain_split():
    """walrus CoreV3 codegen rejects >1 sync wait on the Tile exit drain;
    split the aggregated waits across a chain of drains."""

    def _drain_and_barrier_split(self, tick_clock, wait_clock):
        drain_inst = self.nc.sync.drain()
        wait_clock.add_sem_waits(
            drain_inst.ins, ScopedClock({None: tick_clock.global_clock})
        )
        si = drain_inst.ins.sync_info
        if si is not None and len(si.on_wait) > 1:
            waits = list(si.on_wait)
            updates = list(si.on_update)
            drain_inst.ins.sync_info = mybir.SyncInfo(
                on_wait=waits[:1], on_update=[]
            )
            for i in range(1, len(waits)):
                extra = self.nc.sync.drain()
                extra.ins.sync_info = mybir.SyncInfo(
                    on_wait=waits[i : i + 1],
                    on_update=updates if i + 1 >= len(waits) else [],
                )
        self.nc.all_engine_barrier()
        assert self.sems is not None
        popped = self.nc._tile_sem_poison_stack.pop()
        assert popped is self._sem_poison
        self.nc.clear_and_free_semaphores(list(self.sems.allocated().values()))
        self.nc.all_engine_barrier()

    tile.TileContext._drain_and_barrier = _drain_and_barrier_split


_install_drain_split()


def _prep(x, edge_index):
    """Host-side index preprocessing. Returns (schedule, per-core tensors)."""
    src = np.asarray(edge_index[0], np.int64)
    dst = np.asarray(edge_index[1], np.int64)
    x_bf = np.asarray(x, np.float32).astype(ml_dtypes.bfloat16)

    cores = []
    counts = np.zeros((C, NSEG, NB), np.int64)
    for c in range(C):
        m = (dst >= c * P) & (dst < (c + 1) * P)
        s_c = src[m]
        d_c = dst[m] - c * P
        seg = d_c >> 7
        dloc = d_c & 127
        uniq, inv = np.unique(s_c, return_inverse=True)
        assert len(uniq) <= TABLE_ROWS, len(uniq)
        bucket = (inv >= BCUT).astype(np.int64)
        idx_local = np.where(bucket == 1, inv - BCUT, inv).astype(np.int64)
        assert idx_local.max() < BCUT
        key = bucket * NSEG + seg
        order = np.argsort(key, kind="stable")
        cnt = np.bincount(key, minlength=NB * NSEG).reshape(NB, NSEG).T  # [s, b]
        counts[c] = cnt
        table = np.zeros((TABLE_ROWS, D), ml_dtypes.bfloat16)
        table[: len(uniq)] = x_bf[uniq]
        cores.append(
            dict(table=table, seg=seg, dloc=dloc, idx_local=idx_local,
                 key=key, order=order, dst_local_all=d_c)
        )

    T = -(-counts.max(axis=0) // SEG)  # [NSEG, NB] shared tile counts
    T[:, 0] = np.maximum(T[:, 0], 1)  # every segment has >=1 tile
    tiles_per_seg = T.sum(axis=1)

    # segment-major tile column base: for s: for b
    col_sm = np.zeros((NSEG, NB), np.int64)
    run = 0
    for s in range(NSEG):
        for b in range(NB):
            col_sm[s, b] = run
            run += T[s, b]
    total_tiles = run

    # bucket-major gather column base: for b: for s
    col_bm = np.zeros((NB, NSEG), np.int64)
    run = 0
    for b in range(NB):
        for s in range(NSEG):
            col_bm[b, s] = run
            run += T[s, b]
    total_slots = run * SEG

    # gather chunks: (group, bucket) -> [col_start, col_end) in bucket-major cols
    chunks = []
    for g in range(NG):
        for b in range(NB):
            s0, s1 = g * GSEG, (g + 1) * GSEG
            c0 = col_bm[b, s0]
            c1 = col_bm[b, s1 - 1] + T[s1 - 1, b]
            chunks.append((g, b, int(c0), int(c1)))

    sched = dict(T=T, tiles_per_seg=tiles_per_seg, col_sm=col_sm,
                 col_bm=col_bm, total_tiles=int(total_tiles),
                 total_slots=int(total_slots), chunks=chunks)

    # per-core slot data
    for c in range(C):
        cc = cores[c]
        order = cc["order"]
        key_o = cc["key"][order]
        seg_o = key_o % NSEG
        b_o = key_o // NSEG
        # position within each (b, seg) run
        run_start = np.zeros(NB * NSEG, np.int64)
        cnt_flat = np.bincount(cc["key"], minlength=NB * NSEG)
        run_start[1:] = np.cumsum(cnt_flat)[:-1]
        j = np.arange(len(order)) - run_start[key_o]

        # gather slots (bucket-major)
        idx16 = np.zeros(sched["total_slots"], np.int16)
        gcol = col_bm[b_o, seg_o] + (j >> 7)
        gslot = gcol * SEG + (j & 127)
        idx16[gslot] = cc["idx_local"][order].astype(np.int16)
        idx_wrapped = np.tile(
            idx16.reshape(-1, 16).T, (8, 1)
        )  # [128, total_slots/16]

        # streamed one-hot S (segment-major): S[slot_lane, tile_col*SEG + dst_lane]
        scol = col_sm[seg_o, b_o] + (j >> 7)
        S_np = np.zeros((SEG, sched["total_tiles"] * SEG), ml_dtypes.bfloat16)
        S_np[j & 127, scol * SEG + cc["dloc"][order]] = 1.0

        cnt_node = np.bincount(cc["dst_local_all"], minlength=PPAD)
        cntinv = (1.0 / np.maximum(cnt_node, 1)).astype(np.float32)

        xs = np.asarray(x, np.float32)[c * P : (c + 1) * P]
        x_nm = np.zeros((PPAD, D), np.float32)
        x_nm[:P] = xs
        xT = np.zeros((D, PPAD), np.float32)
        xT[:, :P] = xs.T

        cc["idx_wrapped"] = np.ascontiguousarray(idx_wrapped)
        cc["S"] = S_np
        cc["cntinv"] = np.ascontiguousarray(
            cntinv.reshape(NSEG, SEG).T
        )  # [128, NSEG]
        cc["x_nm"] = x_nm
        cc["xT"] = xT.astype(ml_dtypes.bfloat16)
    return sched, cores


def _build_program(sched, W1, W2, b1, b2, gamma, beta):
    LVL = int(os.environ.get("KLVL", "9"))
    T = sched["T"]
    col_sm = sched["col_sm"]
    col_bm = sched["col_bm"]
    total_tiles = sched["total_tiles"]
    total_slots = sched["total_slots"]
    chunks = sched["chunks"]

    b2_zero = not np.any(b2)
    gamma_one = np.all(gamma == 1.0)
    beta_zero = not np.any(beta)

    nc = bacc.Bacc(
        "TRN2", target_bir_lowering=False, debug=False, num_devices=C,
        num_swdge_queues=int(os.environ.get("KSWQ", "4")),
    )
    t_table = nc.declare_dram_parameter("table", [TABLE_ROWS, D], BF16, isOutput=False)
    t_idx = nc.declare_dram_parameter("idx", [128, total_slots // 16], I16, isOutput=False)
    t_S = nc.declare_dram_parameter("S", [128, total_tiles * SEG], BF16, isOutput=False)
    t_xnm = nc.declare_dram_parameter("xnm", [PPAD, D], F32, isOutput=False)
    t_xT = nc.declare_dram_parameter("xT", [D, PPAD], BF16, isOutput=False)
    t_ci = nc.declare_dram_parameter("cntinv", [128, NSEG], F32, isOutput=False)
    t_W1 = nc.declare_dram_parameter("W1", [D, 64], BF16, isOutput=False)
    t_W2 = nc.declare_dram_parameter("W2", [64, D], BF16, isOutput=False)
    t_b1 = nc.declare_dram_parameter("b1", [64, 1], F32, isOutput=False)
    t_aux = None
    if not (b2_zero and gamma_one and beta_zero):
        # [128, 3*D] f32: b2 / gamma / beta broadcast along partitions
        t_aux = nc.declare_dram_parameter("aux", [128, 3 * D], F32, isOutput=False)
    t_out = nc.declare_dram_parameter("out", [PPAD, D], F32, isOutput=True)

    with tile.TileContext(nc) as tc:
        import contextlib

        ctx = contextlib.ExitStack()
        with ctx:
            singles = ctx.enter_context(tc.tile_pool(name="singles", bufs=1))
            xe_a = ctx.enter_context(tc.tile_pool(name="xe_a", bufs=5))
            xe_b = ctx.enter_context(tc.tile_pool(name="xe_b", bufs=5))
            spool = ctx.enter_context(tc.tile_pool(name="spool", bufs=4))
            xnm_pool = ctx.enter_context(tc.tile_pool(name="xnm", bufs=2))
            xt_pool = ctx.enter_context(tc.tile_pool(name="xt", bufs=2))
            tmp_pool = ctx.enter_context(tc.tile_pool(name="tmp", bufs=4))
            h_pool = ctx.enter_context(tc.tile_pool(name="h", bufs=GSEG + 2))
            o_pool = ctx.enter_context(tc.tile_pool(name="o", bufs=2))
            grp_pool = ctx.enter_context(tc.tile_pool(name="grp", bufs=3))
            ps_agg = ctx.enter_context(
                tc.tile_pool(name="ps_agg", bufs=3, space="PSUM")
            )
            ps_mm1 = ctx.enter_context(
                tc.tile_pool(name="ps_mm1", bufs=2, space="PSUM")
            )
            ps_mm2 = ctx.enter_context(
                tc.tile_pool(name="ps_mm2", bufs=2, space="PSUM")
            )

            KNC = os.environ.get("KNO_CONSTS", "0") == "1"
            w1_t = singles.tile([D, 64], BF16)
            w2_t = singles.tile([64, D], BF16)
            b1_t = singles.tile([64, 1], F32)
            ci_t = singles.tile([128, NSEG], F32)
            idx_t = singles.tile([128, total_slots // 16], I16)
            nc.sync.dma_start(out=idx_t[:], in_=t_idx[:])
            if not KNC:
                nc.sync.dma_start(out=w1_t[:], in_=t_W1[:])
                nc.sync.dma_start(out=w2_t[:], in_=t_W2[:])
                nc.sync.dma_start(out=b1_t[:], in_=t_b1[:])
                nc.sync.dma_start(out=ci_t[:], in_=t_ci[:])
            if t_aux is not None:
                aux_t = singles.tile([128, 3 * D], F32)
                if not KNC:
                    nc.sync.dma_start(out=aux_t[:], in_=t_aux[:])

            eps_t = singles.tile([128, 1], F32)
            if not KNC:
                nc.vector.memset(eps_t[:], LN_EPS)
            nrm2_t = singles.tile([128, NSEG], F32)
            ad_t = singles.tile([128, NSEG], F32)
            relu1 = singles.tile([64, PPAD], BF16)

            # ---- bottleneck MLP, stage 1 (feat-major) ----
            off = 0
            while LVL >= 4 and off < PPAD:
                w = min(MM1_CHUNK, PPAD - off)
                xt_t = xt_pool.tile([D, MM1_CHUNK], BF16, tag="xt")
                nc.sync.dma_start(out=xt_t[:, :w], in_=t_xT[:, off : off + w])
                p1 = ps_mm1.tile([64, MM1_CHUNK], F32, tag="p1")
                nc.tensor.matmul(
                    out=p1[:, :w], lhsT=w1_t[:], rhs=xt_t[:, :w],
                    start=True, stop=True,
                )
                nc.scalar.activation(
                    out=relu1[:, off : off + w], in_=p1[:, :w],
                    func=mybir.ActivationFunctionType.Relu, bias=b1_t[:],
                )
                off += w

            # ---- gathers + per-segment aggregation, grouped ----
            xe_tiles = {}
            for g in range(NG):
                # issue gathers for this group's two bucket chunks
                KGB = os.environ.get("KGB", "")
                for (gg, b, c0, c1) in chunks:
                    if gg != g or LVL < 1:
                        continue
                    if KGB and f"{gg}{b}" not in KGB.split(","):
                        continue
                    nslots = (c1 - c0) * SEG
                    pool = xe_a if b == 0 else xe_b
                    xe_t = pool.tile(
                        [128, (c1 - c0), SEG], BF16, tag=f"xe{b}"
                    )
                    in_ap = t_table[b * BCUT : (b + 1) * BCUT, :]
                    KGM = os.environ.get("KGM", "sp1024")
                    nq = int(os.environ.get("KNQ", "4"))
                    if gg >= NG - 2:
                        # tail groups: per-segment gathers so each segment's
                        # consumers start as soon as its slice lands
                        for s_ in range(gg * GSEG, (gg + 1) * GSEG):
                            cs0 = int(col_bm[b, s_])
                            cs1 = cs0 + int(T[s_, b])
                            if cs1 <= cs0:
                                continue
                            nc.gpsimd.dma_gather(
                                out_ap=xe_t[:, cs0 - c0 : cs1 - c0, :],
                                in_ap=in_ap,
                                idxs_ap=idx_t[:, cs0 * 8 : cs1 * 8],
                                num_idxs=(cs1 - cs0) * SEG,
                                num_idxs_reg=(cs1 - cs0) * SEG,
                                elem_size=D,
                                single_packet=KGM == "sp1024",
                                queue_num=s_ % nq,
                            )
                        xe_tiles[(g, b)] = (xe_t, c0)
                        continue
                    if KGM == "sp1024":
                        qi = g * 3 + b
                        spc = int(os.environ.get("KSPC", "8"))
                        for off in range(0, c1 - c0, spc):
                            w = min(spc, c1 - c0 - off)
                            nc.gpsimd.dma_gather(
                                out_ap=xe_t[:, off : off + w, :],
                                in_ap=in_ap,
                                idxs_ap=idx_t[:, (c0 + off) * 8 : (c0 + off + w) * 8],
                                num_idxs=w * SEG,
                                num_idxs_reg=w * SEG,
                                elem_size=D,
                                single_packet=True,
                                queue_num=qi % nq,
                            )
                            qi += 1
                    else:
                        nc.gpsimd.dma_gather(
                            out_ap=xe_t[:],
                            in_ap=in_ap,
                            idxs_ap=idx_t[:, c0 * 8 : c1 * 8],
                            num_idxs=nslots,
                            num_idxs_reg=nslots,
                            elem_size=D,
                            single_packet=False,
                            queue_num=(g * NB + b) % nq,
                        )
                    xe_tiles[(g, b)] = (xe_t, c0)

                if os.environ.get("KONLY_GATHER", "0") == "1":
                    continue
                xnm_g = xnm_pool.tile([128, GSEG, D], F32, tag="xnm")
                if os.environ.get("KNO_XNM", "0") == "1":
                    nc.vector.memset(xnm_g[:], 0.0)
                else:
                    nc.scalar.dma_start(
                        out=xnm_g[:],
                        in_=t_xnm[g * GROWS : (g + 1) * GROWS, :].rearrange(
                            "(s p) f -> p s f", p=128
                        ),
                    )

                # streamed one-hot S for this group's tiles (seg-major cols)
                gbase = int(col_sm[g * GSEG, 0])
                gend = (
                    int(col_sm[(g + 1) * GSEG, 0]) if g + 1 < NG else total_tiles
                )
                S_g = spool.tile([128, (gend - gbase) * SEG], BF16, tag="S")
                nc.scalar.dma_start(
                    out=S_g[:], in_=t_S[:, gbase * SEG : gend * SEG]
                )

                # aggregation + neg-diff + sq-accum per segment
                for sl in range(GSEG if LVL >= 2 else 0):
                    s = g * GSEG + sl
                    nt = int(sched["tiles_per_seg"][s])
                    cbase = int(col_sm[s, 0])
                    pa = ps_agg.tile([128, SEG], F32, tag="pa")
                    k = 0
                    for b in range(NB):
                        xe_t, c0 = xe_tiles[(g, b)]
                        for tt in range(int(T[s, b])):
                            col = int(col_bm[b, s]) + tt - c0
                            kc = cbase - gbase + k
                            nc.tensor.matmul(
                                out=pa[:],
                                lhsT=S_g[:, kc * SEG : (kc + 1) * SEG],
                                rhs=xe_t[:, col, :],
                                start=(k == 0),
                                stop=(k == nt - 1),
                            )
                            k += 1
                    if LVL < 3:
                        continue
                    negd = tmp_pool.tile([128, D], BF16, tag="negd")
                    nc.vector.scalar_tensor_tensor(
                        out=negd[:],
                        in0=pa[:],
                        scalar=ci_t[:, s : s + 1],
                        in1=xnm_g[:, sl, :],
                        op0=mybir.AluOpType.mult,
                        op1=mybir.AluOpType.subtract,
                    )
                    sq = tmp_pool.tile([128, D], BF16, tag="sq")
                    nc.scalar.activation(
                        out=sq[:],
                        in_=negd[:],
                        func=mybir.ActivationFunctionType.Square,
                        accum_out=nrm2_t[:, s : s + 1],
                    )
                if LVL < 2:
                    for sl in range(GSEG):
                        pass

                # gate: ad = ALPHA * tanh(sqrt(nrm2)) for this group
                gsl = slice(g * GSEG, (g + 1) * GSEG)
                if LVL < 4:
                    o_g = o_pool.tile([128, GSEG, D], F32, tag="og")
                    nc.vector.memset(o_g[:], 0.0)
                    if os.environ.get("KFLAT_OUT", "0") == "1":
                        nc.sync.dma_start(
                            out=t_out[g * GROWS : (g + 1) * GROWS, :].rearrange(
                                "(p s) f -> p (s f)", p=128
                            ),
                            in_=o_g[:],
                        )
                    else:
                        nc.sync.dma_start(
                            out=t_out[g * GROWS : (g + 1) * GROWS, :].rearrange(
                                "(s p) f -> p s f", p=128
                            ),
                            in_=o_g[:],
                        )
                    continue
                tn = grp_pool.tile([128, GSEG], F32, tag="tn")
                nc.scalar.activation(
                    out=tn[:], in_=nrm2_t[:, gsl],
                    func=mybir.ActivationFunctionType.Sqrt,
                )
                nc.scalar.activation(
                    out=ad_t[:, gsl], in_=tn[:],
                    func=mybir.ActivationFunctionType.Tanh,
                )

                # mm2 + residual + LN stats per segment
                mv_g = grp_pool.tile([128, GSEG, 2], F32, tag="mv")
                if LVL < 5:
                    o_g = o_pool.tile([128, GSEG, D], F32, tag="og")
                    nc.vector.memset(o_g[:], 0.0)
                    if os.environ.get("KFLAT_OUT", "0") == "1":
                        nc.sync.dma_start(
                            out=t_out[g * GROWS : (g + 1) * GROWS, :].rearrange(
                                "(p s) f -> p (s f)", p=128
                            ),
                            in_=o_g[:],
                        )
                    else:
                        nc.sync.dma_start(
                            out=t_out[g * GROWS : (g + 1) * GROWS, :].rearrange(
                                "(s p) f -> p s f", p=128
                            ),
                            in_=o_g[:],
                        )
                    continue
                h_list = []
                for sl in range(GSEG):
                    s = g * GSEG + sl
                    p2 = ps_mm2.tile([128, D], F32, tag="p2")
                    nc.tensor.matmul(
                        out=p2[:],
                        lhsT=relu1[:, s * SEG : (s + 1) * SEG],
                        rhs=w2_t[:],
                        start=True,
                        stop=True,
                    )
                    if not b2_zero:
                        nc.vector.tensor_tensor(
                            out=p2[:], in0=p2[:], in1=aux_t[:, 0:D],
                            op=mybir.AluOpType.add,
                        )
                    h_t = h_pool.tile([128, D], F32, tag="h")
                    nc.vector.scalar_tensor_tensor(
                        out=h_t[:],
                        in0=p2[:],
                        scalar=ad_t[:, s : s + 1],
                        in1=xnm_g[:, sl, :],
                        op0=mybir.AluOpType.mult,
                        op1=mybir.AluOpType.add,
                    )
                    st = tmp_pool.tile([128, 6], F32, tag="st")
                    nc.vector.bn_stats(out=st[:], in_=h_t[:])
                    nc.vector.bn_aggr(out=mv_g[:, sl, :], in_=st[:])
                    h_list.append(h_t)

                if LVL < 6:
                    o_g = o_pool.tile([128, GSEG, D], F32, tag="og")
                    nc.vector.memset(o_g[:], 0.0)
                    if os.environ.get("KFLAT_OUT", "0") == "1":
                        nc.sync.dma_start(
                            out=t_out[g * GROWS : (g + 1) * GROWS, :].rearrange(
                                "(p s) f -> p (s f)", p=128
                            ),
                            in_=o_g[:],
                        )
                    else:
                        nc.sync.dma_start(
                            out=t_out[g * GROWS : (g + 1) * GROWS, :].rearrange(
                                "(s p) f -> p s f", p=128
                            ),
                            in_=o_g[:],
                        )
                    continue
                rinv = grp_pool.tile([128, GSEG], F32, tag="rinv")
                nc.scalar.activation(
                    out=rinv[:], in_=mv_g[:, :, 1],
                    func=mybir.ActivationFunctionType.Sqrt, bias=eps_t[:],
                )
                nc.vector.reciprocal(out=rinv[:], in_=rinv[:])
                mur = grp_pool.tile([128, GSEG], F32, tag="mur")
                nc.vector.tensor_tensor(
                    out=mur[:], in0=mv_g[:, :, 0], in1=rinv[:],
                    op=mybir.AluOpType.mult,
                )

                o_g = o_pool.tile([128, GSEG, D], F32, tag="og")
                for sl in range(GSEG):
                    nc.vector.scalar_tensor_tensor(
                        out=o_g[:, sl, :],
                        in0=h_list[sl][:],
                        scalar=rinv[:, sl : sl + 1],
                        in1=mur[:, sl : sl + 1].to_broadcast([128, D]),
                        op0=mybir.AluOpType.mult,
                        op1=mybir.AluOpType.subtract,
                    )
                    if not gamma_one:
                        nc.vector.tensor_tensor(
                            out=o_g[:, sl, :], in0=o_g[:, sl, :],
                            in1=aux_t[:, D : 2 * D], op=mybir.AluOpType.mult,
                        )
                    if not beta_zero:
                        nc.vector.tensor_tensor(
                            out=o_g[:, sl, :], in0=o_g[:, sl, :],
                            in1=aux_t[:, 2 * D : 3 * D], op=mybir.AluOpType.add,
                        )
                nc.sync.dma_start(
                    out=t_out[g * GROWS : (g + 1) * GROWS, :].rearrange(
                        "(s p) f -> p s f", p=128
                    ),
                    in_=o_g[:],
                )
    return nc


def kernel(**inputs) -> np.ndarray:
    x = np.asarray(inputs["x"], np.float32)
    edge_index = np.asarray(inputs["edge_index"])
    W1 = np.asarray(inputs["W1"], np.float32)
    b1 = np.asarray(inputs["b1"], np.float32)
    W2 = np.asarray(inputs["W2"], np.float32)
    b2 = np.asarray(inputs["b2"], np.float32)
    gamma = np.asarray(inputs["gamma"], np.float32)
    beta = np.asarray(inputs["beta"], np.float32)

    sched, cores = _prep(x, edge_index)
    nc = _build_program(sched, W1, W2, b1, b2, gamma, beta)

    w1_np = W1.astype(ml_dtypes.bfloat16)
    w2_np = (W2 * ALPHA).astype(ml_dtypes.bfloat16)
    b1_np = b1.reshape(64, 1).astype(np.float32)
    need_aux = not (
        (not np.any(b2)) and np.all(gamma == 1.0) and (not np.any(beta))
    )
    if need_aux:
        aux_np = np.concatenate(
            [np.tile(v, (128, 1)) for v in (b2 * ALPHA, gamma, beta)], axis=1
        ).astype(np.float32)

    in_maps = []
    for c in range(C):
        cc = cores[c]
        m = {
            "table": cc["table"],
            "idx": cc["idx_wrapped"],
            "S": cc["S"],
            "xnm": cc["x_nm"],
            "xT": cc["xT"],
            "cntinv": cc["cntinv"],
            "W1": w1_np,
            "W2": w2_np,
            "b1": b1_np,
        }
        if need_aux:
            m["aux"] = aux_np
        in_maps.append(m)

    trace = os.environ.get("KERNEL_TRACE", "0") == "1"
    nc.finalize()
    res = run_bass_kernel_spmd(
        nc, in_maps, core_ids=list(range(C)), trace=trace
    )
    if trace and res.exec_time_ns is not None:
        print(f"HW exec time: {res.exec_time_ns} ns")
        kernel.last_exec_time_ns = res.exec_time_ns

    out = np.empty((N, D), np.float32)
    for c in range(C):
        out[c * P : (c + 1) * P] = res.results[c]["out"][:P]
    return out


if __name__ == "__main__":
    # quick self-test against reference
    os.environ.setdefault("KERNEL_TRACE", "1")
    sys.path.insert(0, os.path.dirname(os.path.abspath(__file__)))
    import reference

    inputs = reference.setup_inputs()
    inputs = {k: np.asarray(v) for k, v in inputs.items()}
    got = kernel(**inputs)
    print("out", got.shape, got.dtype)



# revision 15
# speedup vs baseline: 1.1302x; 1.1302x over previous
"""DiffuseEnhancer (GNN mean-aggregation + gated MLP + LayerNorm) on 8 TRN2
NeuronCores via Bass/Tile.

Strategy (SPMD, one program for all 8 cores):
- Nodes sharded by destination: core c owns dst rows [c*12500, (c+1)*12500).
- Edges partitioned by destination core; per core, grouped by 128-dst
  segments. Edge-source features are DMA-gathered (dma_gather, int16
  indices) from a per-core compacted bf16 node table: the core's unique
  source nodes, split into two <=32768-row buckets so indices fit int16.
- Gathers run on 4 SWDGE queues (num_swdge_queues=4) as 1024-row
  single-packet sub-gathers striped round-robin over queue_num 0-3;
  this roughly halves the Q7 descriptor-emission wall (~7.6 -> ~4.2
  ns/row) that otherwise serializes the whole kernel.
- Mean aggregation per segment via TensorE: host-built one-hot S
  matrices ([slots, 128 dst] bf16, streamed from DRAM per group) times
  gathered features, accumulated in PSUM -> msg[128 dst x 128 feat].
  Streaming S replaces the on-device is_equal build, whose broadcast
  operands forced DVE 1x mode (~103us).
- Epilogue per segment fuses: mean-scale + subtract (scalar_tensor_tensor,
  reads PSUM), squared-norm (ACT Square + accum), tanh gate, bottleneck
  MLP (two matmuls), residual assembly, LayerNorm (bn_stats/bn_aggr).

The tile/bucket schedule is shared across cores (max over cores, padded
slots gather throwaway rows and carry all-zero S columns), so a single
NEFF serves all 8 cores; per-core data lives in the input tensors.
"""

import os
import sys

for _p in ("/opt/trn_rl_repo", "/root/.axon_site/_ro/trn_rl_repo"):
    if os.path.isdir(_p) and _p not in sys.path:
        sys.path.insert(0, _p)

import numpy as np
import ml_dtypes

# graceful degradation if the NTFF profile hook module is absent
try:
    import antenv.axon_hooks  # noqa: F401
except ImportError:
    import types

    _m = types.ModuleType("antenv.axon_hooks")
    _m._HOOK = None
    _m.set_axon_ntff_profile_hook = lambda h: setattr(_m, "_HOOK", h)
    _m.get_axon_ntff_profile_hook = lambda: _m._HOOK
    sys.modules["antenv.axon_hooks"] = _m

import concourse.bass as bass
import concourse.bacc as bacc
import concourse.tile as tile
from concourse import mybir
from concourse.bass_utils import run_bass_kernel_spmd
from concourse.vector_clock import ScopedClock

ALPHA = 0.2
LN_EPS = 1e-5

N, D, C = 100000, 128, 8
P = N // C            # 12500 nodes per core
SEG = 128
NSEG = (P + SEG - 1) // SEG       # 98
PPAD = NSEG * SEG                 # 12544
NB = 2                            # src buckets per core
BCUT = 32768                      # bucket A = first 32768 unique srcs
TABLE_ROWS = 2 * BCUT             # fixed per-core gather table height
GSEG = 7                          # segments per gather/epilogue group
NG = NSEG // GSEG                 # 14
GROWS = GSEG * SEG                # 1792
MM1_CHUNK = 512
SENTINEL = 255.0

BF16 = mybir.dt.bfloat16
F32 = mybir.dt.float32
I16 = mybir.dt.int16


def _install_drain_split():
    """walrus CoreV3 codegen rejects >1 sync wait on the Tile exit drain;
    split the aggregated waits across a chain of drains."""

    def _drain_and_barrier_split(self, tick_clock, wait_clock):
        drain_inst = self.nc.sync.drain()
        wait_clock.add_sem_waits(
            drain_inst.ins, ScopedClock({None: tick_clock.global_clock})
        )
        si = drain_inst.ins.sync_info
        if si is not None and len(si.on_wait) > 1:
            waits = list(si.on_wait)
            updates = list(si.on_update)
            drain_inst.ins.sync_info = mybir.SyncInfo(
                on_wait=waits[:1], on_update=[]
            )
            for i in range(1, len(waits)):
                extra = self.nc.sync.drain()
                extra.ins.sync_info = mybir.SyncInfo(
                    on_wait=waits[i : i + 1],
                    on_update=updates if i + 1 >= len(waits) else [],
                )
        self.nc.all_engine_barrier()
        assert self.sems is not None
        popped = self.nc._tile_sem_poison_stack.pop()
        assert popped is self._sem_poison
        self.nc.clear_and_free_semaphores(list(self.sems.allocated().values()))
        self.nc.all_engine_barrier()

    tile.TileContext._drain_and_barrier = _drain_and_barrier_split


_install_drain_split()


def _prep(x, edge_index):
    """Host-side index preprocessing. Returns (schedule, per-core tensors)."""
    src = np.asarray(edge_index[0], np.int64)
    dst = np.asarray(edge_index[1], np.int64)
    x_bf = np.asarray(x, np.float32).astype(ml_dtypes.bfloat16)

    cores = []
    counts = np.zeros((C, NSEG, NB), np.int64)
    for c in range(C):
        m = (dst >= c * P) & (dst < (c + 1) * P)
        s_c = src[m]
        d_c = dst[m] - c * P
        seg = d_c >> 7
        dloc = d_c & 127
        uniq, inv = np.unique(s_c, return_inverse=True)
        assert len(uniq) <= TABLE_ROWS, len(uniq)
        bucket = (inv >= BCUT).astype(np.int64)
        idx_local = np.where(bucket == 1, inv - BCUT, inv).astype(np.int64)
        assert idx_local.max() < BCUT
        key = bucket * NSEG + seg
        order = np.argsort(key, kind="stable")
        cnt = np.bincount(key, minlength=NB * NSEG).reshape(NB, NSEG).T  # [s, b]
        counts[c] = cnt
        table = np.zeros((TABLE_ROWS, D), ml_dtypes.bfloat16)
        table[: len(uniq)] = x_bf[uniq]
        cores.append(
            dict(table=table, seg=seg, dloc=dloc, idx_local=idx_local,
                 key=key, order=order, dst_local_all=d_c)
        )

    T = -(-counts.max(axis=0) // SEG)  # [NSEG, NB] shared tile counts
    T[:, 0] = np.maximum(T[:, 0], 1)  # every segment has >=1 tile
    tiles_per_seg = T.sum(axis=1)

    # segment-major tile column base: for s: for b
    col_sm = np.zeros((NSEG, NB), np.int64)
    run = 0
    for s in range(NSEG):
        for b in range(NB):
            col_sm[s, b] = run
            run += T[s, b]
    total_tiles = run

    # bucket-major gather column base: for b: for s
    col_bm = np.zeros((NB, NSEG), np.int64)
    run = 0
    for b in range(NB):
        for s in range(NSEG):
            col_bm[b, s] = run
            run += T[s, b]
    total_slots = run * SEG

    # gather chunks: (group, bucket) -> [col_start, col_end) in bucket-major cols
    chunks = []
    for g in range(NG):
        for b in range(NB):
            s0, s1 = g * GSEG, (g + 1) * GSEG
            c0 = col_bm[b, s0]
            c1 = col_bm[b, s1 - 1] + T[s1 - 1, b]
            chunks.append((g, b, int(c0), int(c1)))

    sched = dict(T=T, tiles_per_seg=tiles_per_seg, col_sm=col_sm,
                 col_bm=col_bm, total_tiles=int(total_tiles),
                 total_slots=int(total_slots), chunks=chunks)

    # per-core slot data
    for c in range(C):
        cc = cores[c]
        order = cc["order"]
        key_o = cc["key"][order]
        seg_o = key_o % NSEG
        b_o = key_o // NSEG
        # position within each (b, seg) run
        run_start = np.zeros(NB * NSEG, np.int64)
        cnt_flat = np.bincount(cc["key"], minlength=NB * NSEG)
        run_start[1:] = np.cumsum(cnt_flat)[:-1]
        j = np.arange(len(order)) - run_start[key_o]

        # gather slots (bucket-major)
        idx16 = np.zeros(sched["total_slots"], np.int16)
        gcol = col_bm[b_o, seg_o] + (j >> 7)
        gslot = gcol * SEG + (j & 127)
        idx16[gslot] = cc["idx_local"][order].astype(np.int16)
        idx_wrapped = np.tile(
            idx16.reshape(-1, 16).T, (8, 1)
        )  # [128, total_slots/16]

        # streamed one-hot S (segment-major): S[slot_lane, tile_col*SEG + dst_lane]
        scol = col_sm[seg_o, b_o] + (j >> 7)
        S_np = np.zeros((SEG, sched["total_tiles"] * SEG), ml_dtypes.bfloat16)
        S_np[j & 127, scol * SEG + cc["dloc"][order]] = 1.0

        cnt_node = np.bincount(cc["dst_local_all"], minlength=PPAD)
        cntinv = (1.0 / np.maximum(cnt_node, 1)).astype(np.float32)

        xs = np.asarray(x, np.float32)[c * P : (c + 1) * P]
        x_nm = np.zeros((PPAD, D), np.float32)
        x_nm[:P] = xs
        xT = np.zeros((D, PPAD), np.float32)
        xT[:, :P] = xs.T

        cc["idx_wrapped"] = np.ascontiguousarray(idx_wrapped)
        cc["S"] = S_np
        cc["cntinv"] = np.ascontiguousarray(
            cntinv.reshape(NSEG, SEG).T
        )  # [128, NSEG]
        cc["x_nm"] = x_nm
        cc["xT"] = xT.astype(ml_dtypes.bfloat16)
    return sched, cores


def _build_program(sched, W1, W2, b1, b2, gamma, beta):
    LVL = int(os.environ.get("KLVL", "9"))
    T = sched["T"]
    col_sm = sched["col_sm"]
    col_bm = sched["col_bm"]
    total_tiles = sched["total_tiles"]
    total_slots = sched["total_slots"]
    chunks = sched["chunks"]

    b2_zero = not np.any(b2)
    gamma_one = np.all(gamma == 1.0)
    beta_zero = not np.any(beta)

    nc = bacc.Bacc(
        "TRN2", target_bir_lowering=False, debug=False, num_devices=C,
        num_swdge_queues=int(os.environ.get("KSWQ", "4")),
    )
    t_table = nc.declare_dram_parameter("table", [TABLE_ROWS, D], BF16, isOutput=False)
    t_idx = nc.declare_dram_parameter("idx", [128, total_slots // 16], I16, isOutput=False)
    t_S = nc.declare_dram_parameter("S", [128, total_tiles * SEG], BF16, isOutput=False)
    t_xnm = nc.declare_dram_parameter("xnm", [PPAD, D], F32, isOutput=False)
    t_xT = nc.declare_dram_parameter("xT", [D, PPAD], BF16, isOutput=False)
    t_ci = nc.declare_dram_parameter("cntinv", [128, NSEG], F32, isOutput=False)
    t_W1 = nc.declare_dram_parameter("W1", [D, 64], BF16, isOutput=False)
    t_W2 = nc.declare_dram_parameter("W2", [64, D], BF16, isOutput=False)
    t_b1 = nc.declare_dram_parameter("b1", [64, 1], F32, isOutput=False)
    t_aux = None
    if not (b2_zero and gamma_one and beta_zero):
        # [128, 3*D] f32: b2 / gamma / beta broadcast along partitions
        t_aux = nc.declare_dram_parameter("aux", [128, 3 * D], F32, isOutput=False)
    t_out = nc.declare_dram_parameter("out", [PPAD, D], F32, isOutput=True)

    with tile.TileContext(nc) as tc:
        import contextlib

        ctx = contextlib.ExitStack()
        with ctx:
            singles = ctx.enter_context(tc.tile_pool(name="singles", bufs=1))
            xe_a = ctx.enter_context(tc.tile_pool(name="xe_a", bufs=5))
            xe_b = ctx.enter_context(tc.tile_pool(name="xe_b", bufs=5))
            spool = ctx.enter_context(tc.tile_pool(name="spool", bufs=3))
            xnm_pool = ctx.enter_context(tc.tile_pool(name="xnm", bufs=2))
            xt_pool = ctx.enter_context(tc.tile_pool(name="xt", bufs=2))
            tmp_pool = ctx.enter_context(tc.tile_pool(name="tmp", bufs=4))
            h_pool = ctx.enter_context(tc.tile_pool(name="h", bufs=GSEG + 2))
            o_pool = ctx.enter_context(tc.tile_pool(name="o", bufs=2))
            grp_pool = ctx.enter_context(tc.tile_pool(name="grp", bufs=3))
            ps_agg = ctx.enter_context(
                tc.tile_pool(name="ps_agg", bufs=3, space="PSUM")
            )
            ps_mm1 = ctx.enter_context(
                tc.tile_pool(name="ps_mm1", bufs=2, space="PSUM")
            )
            ps_mm2 = ctx.enter_context(
                tc.tile_pool(name="ps_mm2", bufs=2, space="PSUM")
            )

            KNC = os.environ.get("KNO_CONSTS", "0") == "1"
            w1_t = singles.tile([D, 64], BF16)
            w2_t = singles.tile([64, D], BF16)
            b1_t = singles.tile([64, 1], F32)
            ci_t = singles.tile([128, NSEG], F32)
            idx_t = singles.tile([128, total_slots // 16], I16)
            nc.sync.dma_start(out=idx_t[:], in_=t_idx[:])
            if not KNC:
                nc.sync.dma_start(out=w1_t[:], in_=t_W1[:])
                nc.sync.dma_start(out=w2_t[:], in_=t_W2[:])
                nc.sync.dma_start(out=b1_t[:], in_=t_b1[:])
                nc.sync.dma_start(out=ci_t[:], in_=t_ci[:])
            if t_aux is not None:
                aux_t = singles.tile([128, 3 * D], F32)
                if not KNC:
                    nc.sync.dma_start(out=aux_t[:], in_=t_aux[:])

            eps_t = singles.tile([128, 1], F32)
            if not KNC:
                nc.vector.memset(eps_t[:], LN_EPS)
            nrm2_t = singles.tile([128, NSEG], F32)
            ad_t = singles.tile([128, NSEG], F32)
            relu1 = singles.tile([64, PPAD], BF16)

            # ---- bottleneck MLP, stage 1 (feat-major) ----
            off = 0
            while LVL >= 4 and off < PPAD:
                w = min(MM1_CHUNK, PPAD - off)
                xt_t = xt_pool.tile([D, MM1_CHUNK], BF16, tag="xt")
                nc.sync.dma_start(out=xt_t[:, :w], in_=t_xT[:, off : off + w])
                p1 = ps_mm1.tile([64, MM1_CHUNK], F32, tag="p1")
                nc.tensor.matmul(
                    out=p1[:, :w], lhsT=w1_t[:], rhs=xt_t[:, :w],
                    start=True, stop=True,
                )
                nc.scalar.activation(
                    out=relu1[:, off : off + w], in_=p1[:, :w],
                    func=mybir.ActivationFunctionType.Relu, bias=b1_t[:],
                )
                off += w

            # ---- gathers + per-segment aggregation, grouped ----
            xe_tiles = {}
            for g in range(NG):
                # issue gathers for this group's two bucket chunks
                KGB = os.environ.get("KGB", "")
                for (gg, b, c0, c1) in chunks:
                    if gg != g or LVL < 1:
                        continue
                    if KGB and f"{gg}{b}" not in KGB.split(","):
                        continue
                    nslots = (c1 - c0) * SEG
                    pool = xe_a if b == 0 else xe_b
                    xe_t = pool.tile(
                        [128, (c1 - c0), SEG], BF16, tag=f"xe{b}"
                    )
                    in_ap = t_table[b * BCUT : (b + 1) * BCUT, :]
                    KGM = os.environ.get("KGM", "sp1024")
                    nq = int(os.environ.get("KNQ", "4"))
                    if gg >= NG - 2:
                        # tail groups: per-segment gathers so each segment's
                        # consumers start as soon as its slice lands
                        for s_ in range(gg * GSEG, (gg + 1) * GSEG):
                            cs0 = int(col_bm[b, s_])
                            cs1 = cs0 + int(T[s_, b])
                            if cs1 <= cs0:
                                continue
                            nc.gpsimd.dma_gather(
                                out_ap=xe_t[:, cs0 - c0 : cs1 - c0, :],
                                in_ap=in_ap,
                                idxs_ap=idx_t[:, cs0 * 8 : cs1 * 8],
                                num_idxs=(cs1 - cs0) * SEG,
                                num_idxs_reg=(cs1 - cs0) * SEG,
                                elem_size=D,
                                single_packet=KGM == "sp1024",
                                queue_num=s_ % nq,
                            )
                        xe_tiles[(g, b)] = (xe_t, c0)
                        continue
                    if KGM == "sp1024":
                        qi = g * 3 + b
                        spc = int(os.environ.get("KSPC", "8"))
                        for off in range(0, c1 - c0, spc):
                            w = min(spc, c1 - c0 - off)
                            nc.gpsimd.dma_gather(
                                out_ap=xe_t[:, off : off + w, :],
                                in_ap=in_ap,
                                idxs_ap=idx_t[:, (c0 + off) * 8 : (c0 + off + w) * 8],
                                num_idxs=w * SEG,
                                num_idxs_reg=w * SEG,
                                elem_size=D,
                                single_packet=True,
                                queue_num=qi % nq,
                            )
                            qi += 1
                    else:
                        nc.gpsimd.dma_gather(
                            out_ap=xe_t[:],
                            in_ap=in_ap,
                            idxs_ap=idx_t[:, c0 * 8 : c1 * 8],
                            num_idxs=nslots,
                            num_idxs_reg=nslots,
                            elem_size=D,
                            single_packet=False,
                            queue_num=(g * NB + b) % nq,
                        )
                    xe_tiles[(g, b)] = (xe_t, c0)

                if os.environ.get("KONLY_GATHER", "0") == "1":
                    continue
                xnm_g = xnm_pool.tile([128, GSEG, D], F32, tag="xnm")
                if os.environ.get("KNO_XNM", "0") == "1":
                    nc.vector.memset(xnm_g[:], 0.0)
                else:
                    nc.sync.dma_start(
                        out=xnm_g[:],
                        in_=t_xnm[g * GROWS : (g + 1) * GROWS, :].rearrange(
                            "(s p) f -> p s f", p=128
                        ),
                    )

                # streamed one-hot S for this group's tiles (seg-major cols)
                gbase = int(col_sm[g * GSEG, 0])
                gend = (
                    int(col_sm[(g + 1) * GSEG, 0]) if g + 1 < NG else total_tiles
                )
                S_g = spool.tile([128, (gend - gbase) * SEG], BF16, tag="S")
                nc.sync.dma_start(
                    out=S_g[:], in_=t_S[:, gbase * SEG : gend * SEG]
                )

                # aggregation + neg-diff + sq-accum per segment
                for sl in range(GSEG if LVL >= 2 else 0):
                    s = g * GSEG + sl
                    nt = int(sched["tiles_per_seg"][s])
                    cbase = int(col_sm[s, 0])
                    pa = ps_agg.tile([128, SEG], F32, tag="pa")
                    k = 0
                    for b in range(NB):
                        xe_t, c0 = xe_tiles[(g, b)]
                        for tt in range(int(T[s, b])):
                            col = int(col_bm[b, s]) + tt - c0
                            kc = cbase - gbase + k
                            nc.tensor.matmul(
                                out=pa[:],
                                lhsT=S_g[:, kc * SEG : (kc + 1) * SEG],
                                rhs=xe_t[:, col, :],
                                start=(k == 0),
                                stop=(k == nt - 1),
                            )
                            k += 1
                    if LVL < 3:
                        continue
                    negd = tmp_pool.tile([128, D], BF16, tag="negd")
                    nc.vector.scalar_tensor_tensor(
                        out=negd[:],
                        in0=pa[:],
                        scalar=ci_t[:, s : s + 1],
                        in1=xnm_g[:, sl, :],
                        op0=mybir.AluOpType.mult,
                        op1=mybir.AluOpType.subtract,
                    )
                    sq = tmp_pool.tile([128, D], BF16, tag="sq")
                    nc.scalar.activation(
                        out=sq[:],
                        in_=negd[:],
                        func=mybir.ActivationFunctionType.Square,
                        accum_out=nrm2_t[:, s : s + 1],
                    )
                if LVL < 2:
                    for sl in range(GSEG):
                        pass

                # gate: ad = ALPHA * tanh(sqrt(nrm2)) for this group
                gsl = slice(g * GSEG, (g + 1) * GSEG)
                if LVL < 4:
                    o_g = o_pool.tile([128, GSEG, D], F32, tag="og")
                    nc.vector.memset(o_g[:], 0.0)
                    if os.environ.get("KFLAT_OUT", "0") == "1":
                        nc.sync.dma_start(
                            out=t_out[g * GROWS : (g + 1) * GROWS, :].rearrange(
                                "(p s) f -> p (s f)", p=128
                            ),
                            in_=o_g[:],
                        )
                    else:
                        nc.sync.dma_start(
                            out=t_out[g * GROWS : (g + 1) * GROWS, :].rearrange(
                                "(s p) f -> p s f", p=128
                            ),
                            in_=o_g[:],
                        )
                    continue
                tn = grp_pool.tile([128, GSEG], F32, tag="tn")
                nc.scalar.activation(
                    out=tn[:], in_=nrm2_t[:, gsl],
                    func=mybir.ActivationFunctionType.Sqrt,
                )
                nc.scalar.activation(
                    out=ad_t[:, gsl], in_=tn[:],
                    func=mybir.ActivationFunctionType.Tanh,
                )

                # mm2 + residual + LN stats per segment
                mv_g = grp_pool.tile([128, GSEG, 2], F32, tag="mv")
                if LVL < 5:
                    o_g = o_pool.tile([128, GSEG, D], F32, tag="og")
                    nc.vector.memset(o_g[:], 0.0)
                    if os.environ.get("KFLAT_OUT", "0") == "1":
                        nc.sync.dma_start(
                            out=t_out[g * GROWS : (g + 1) * GROWS, :].rearrange(
                                "(p s) f -> p (s f)", p=128
                            ),
                            in_=o_g[:],
                        )
                    else:
                        nc.sync.dma_start(
                            out=t_out[g * GROWS : (g + 1) * GROWS, :].rearrange(
                                "(s p) f -> p s f", p=128
                            ),
                            in_=o_g[:],
                        )
                    continue
                h_list = []
                for sl in range(GSEG):
                    s = g * GSEG + sl
                    p2 = ps_mm2.tile([128, D], F32, tag="p2")
                    nc.tensor.matmul(
                        out=p2[:],
                        lhsT=relu1[:, s * SEG : (s + 1) * SEG],
                        rhs=w2_t[:],
                        start=True,
                        stop=True,
                    )
                    if not b2_zero:
                        nc.vector.tensor_tensor(
                            out=p2[:], in0=p2[:], in1=aux_t[:, 0:D],
                            op=mybir.AluOpType.add,
                        )
                    h_t = h_pool.tile([128, D], F32, tag="h")
                    nc.vector.scalar_tensor_tensor(
                        out=h_t[:],
                        in0=p2[:],
                        scalar=ad_t[:, s : s + 1],
                        in1=xnm_g[:, sl, :],
                        op0=mybir.AluOpType.mult,
                        op1=mybir.AluOpType.add,
                    )
                    st = tmp_pool.tile([128, 6], F32, tag="st")
                    nc.vector.bn_stats(out=st[:], in_=h_t[:])
                    nc.vector.bn_aggr(out=mv_g[:, sl, :], in_=st[:])
                    h_list.append(h_t)

                if LVL < 6:
                    o_g = o_pool.tile([128, GSEG, D], F32, tag="og")
                    nc.vector.memset(o_g[:], 0.0)
                    if os.environ.get("KFLAT_OUT", "0") == "1":
                        nc.sync.dma_start(
                            out=t_out[g * GROWS : (g + 1) * GROWS, :].rearrange(
                                "(p s) f -> p (s f)", p=128
                            ),
                            in_=o_g[:],
                        )
                    else:
                        nc.sync.dma_start(
                            out=t_out[g * GROWS : (g + 1) * GROWS, :].rearrange(
                                "(s p) f -> p s f", p=128
                            ),
                            in_=o_g[:],
                        )
                    continue
                rinv = grp_pool.tile([128, GSEG], F32, tag="rinv")
                nc.scalar.activation(
                    out=rinv[:], in_=mv_g[:, :, 1],
                    func=mybir.ActivationFunctionType.Sqrt, bias=eps_t[:],
                )
                nc.vector.reciprocal(out=rinv[:], in_=rinv[:])
                mur = grp_pool.tile([128, GSEG], F32, tag="mur")
                nc.vector.tensor_tensor(
                    out=mur[:], in0=mv_g[:, :, 0], in1=rinv[:],
                    op=mybir.AluOpType.mult,
                )

                o_g = o_pool.tile([128, GSEG, D], F32, tag="og")
                for sl in range(GSEG):
                    nc.vector.scalar_tensor_tensor(
                        out=o_g[:, sl, :],
                        in0=h_list[sl][:],
                        scalar=rinv[:, sl : sl + 1],
                        in1=mur[:, sl : sl + 1].to_broadcast([128, D]),
                        op0=mybir.AluOpType.mult,
                        op1=mybir.AluOpType.subtract,
                    )
                    if not gamma_one:
                        nc.vector.tensor_tensor(
                            out=o_g[:, sl, :], in0=o_g[:, sl, :],
                            in1=aux_t[:, D : 2 * D], op=mybir.AluOpType.mult,
                        )
                    if not beta_zero:
                        nc.vector.tensor_tensor(
                            out=o_g[:, sl, :], in0=o_g[:, sl, :],
                            in1=aux_t[:, 2 * D : 3 * D], op=mybir.AluOpType.add,
                        )
                nc.sync.dma_start(
                    out=t_out[g * GROWS : (g + 1) * GROWS, :].rearrange(
                        "(s p) f -> p s f", p=128
                    ),
                    in_=o_g[:],
                )
    return nc


def kernel(**inputs) -> np.ndarray:
    x = np.asarray(inputs["x"], np.float32)
    edge_index = np.asarray(inputs["edge_index"])
    W1 = np.asarray(inputs["W1"], np.float32)
    b1 = np.asarray(inputs["b1"], np.float32)
    W2 = np.asarray(inputs["W2"], np.float32)
    b2 = np.asarray(inputs["b2"], np.float32)
    gamma = np.asarray(inputs["gamma"], np.float32)
    beta = np.asarray(inputs["beta"], np.float32)

    sched, cores = _prep(x, edge_index)
    nc = _build_program(sched, W1, W2, b1, b2, gamma, beta)

    w1_np = W1.astype(ml_dtypes.bfloat16)
    w2_np = (W2 * ALPHA).astype(ml_dtypes.bfloat16)
    b1_np = b1.reshape(64, 1).astype(np.float32)
    need_aux = not (
        (not np.any(b2)) and np.all(gamma == 1.0) and (not np.any(beta))
    )
    if need_aux:
        aux_np = np.concatenate(
            [np.tile(v, (128, 1)) for v in (b2 * ALPHA, gamma, beta)], axis=1
        ).astype(np.float32)

    in_maps = []
    for c in range(C):
        cc = cores[c]
        m = {
            "table": cc["table"],
            "idx": cc["idx_wrapped"],
            "S": cc["S"],
            "xnm": cc["x_nm"],
            "xT": cc["xT"],
            "cntinv": cc["cntinv"],
            "W1": w1_np,
            "W2": w2_np,
            "b1": b1_np,
        }
        if need_aux:
            m["aux"] = aux_np
        in_maps.append(m)

    trace = os.environ.get("KERNEL_TRACE", "0") == "1"
    nc.finalize()
    res = run_bass_kernel_spmd(
        nc, in_maps, core_ids=list(range(C)), trace=trace
    )
    if trace and res.exec_time_ns is not None:
        print(f"HW exec time: {res.exec_time_ns} ns")
        kernel.last_exec_time_ns = res.exec_time_ns

    out = np.empty((N, D), np.float32)
    for c in range(C):
        out[c * P : (c + 1) * P] = res.results[c]["out"][:P]
    return out


if __name__ == "__main__":
    # quick self-test against reference
    os.environ.setdefault("KERNEL_TRACE", "1")
    sys.path.insert(0, os.path.dirname(os.path.abspath(__file__)))
    import reference

    inputs = reference.setup_inputs()
    inputs = {k: np.asarray(v) for k, v in inputs.items()}
    got = kernel(**inputs)
    print("out", got.shape, got.dtype)

